# revision 18
# baseline (speedup 1.0000x reference)
"""FP8 GEMM kernel for Trainium2 (8 NeuronCores, SPMD data-parallel over tokens).

Computes: out = fp16( fp32( e5m2(x) @ e4m3(weight.T) ) + bias )
  x      [4, 4096, 4096] fp16
  weight [4096, 4096]    fp16  (out_features, in_features)
  bias   [4096]          fp16
  out    [4, 4096, 4096] fp16

Sharding: token dim (B*S = 16384) split across 8 cores (2048 rows each);
weight + bias replicated. No collectives; host concatenates the outputs.

The host quantizes both operands to fp8 (ml_dtypes RNE — bit-identical to
the reference's own jnp casts) and pre-packs them into per-tile K-major
blocks (`[tile][ki=128][ko=32][free]`), so every device load is a plain
contiguous fp8 HWDGE DMA — no in-flight cast, half the bytes of an fp16
stream.

Per-core kernel (~462us against the 442us fp8 DoubleRow streaming floor):
 - DoubleRow fp8 matmuls (K=256/instr, moving free dim 2x512 at the
   ~216ns/MM streaming floor) accumulate fp32 into PSUM; x8 (8MB) stays
   resident, w8 n-tiles stream through a 3-deep pool.
 - Ramp: early DMA arrival is completion-latency-paced (~2.2us per
   dma_start per queue regardless of size), so the ramp uses few, large
   chunks: x0+x1 ride in one host-packed combined block (2 DMAs), w0 in 3
   chunks, and the first 4 groups run k-chunk-outer interleaved over 2
   PSUM banks so the PE never outruns the arrival chain.
 - Remaining x tiles alternate across the sync/scalar HWDGE queues ahead
   of the bias broadcast (psum depth 8 lets the first eviction wait for
   bias until ~6 groups in); weight tiles + output stores ride sync.
 - Bias add fused into the PSUM eviction on DVE (its only job). The final
   group's eviction/store is split across both queues to overlap the
   closing HBM-write receipt.
"""

import sys

if "/opt/trn_rl_repo" not in sys.path:
    sys.path.insert(0, "/opt/trn_rl_repo")

import ml_dtypes
import numpy as np

B, S, DIN, DOUT = 4, 4096, 4096, 4096
NCORES = 8
M_TOTAL = B * S              # 16384
M_LOC = M_TOTAL // NCORES    # 2048
P = 128
M_TILES = M_LOC // P         # 16 m-tiles of 128 rows
N_TILE = 512
N_TILES = DOUT // N_TILE     # 8
K_SUB = DIN // P             # 32 k-subtiles of 128
K_CHUNKS = K_SUB // 2        # 16 DoubleRow chunks of 256
WARM_M = 4                   # m-groups interleaved during the w0 ramp

_cached_nc = None


def _build():
    global _cached_nc
    if _cached_nc is not None:
        return _cached_nc

    import concourse.mybir as mybir
    import concourse.tile as tile
    from concourse import bacc

    nc = bacc.Bacc("TRN2", target_bir_lowering=False, debug=False,
                   num_devices=NCORES)

    # host-packed fp8 K-major tile blocks (see make_in_maps)
    xd01 = nc.dram_tensor("xd01", [P, WARM_M, K_SUB, P], mybir.dt.float8e5,
                          kind="ExternalInput")
    xd = nc.dram_tensor("xd", [M_TILES, P, K_SUB, P], mybir.dt.float8e5,
                        kind="ExternalInput")
    wd = nc.dram_tensor("wd", [N_TILES, P, K_SUB, N_TILE], mybir.dt.float8e4,
                        kind="ExternalInput")
    bvec = nc.dram_tensor("bvec", [DOUT], mybir.dt.float16,
                          kind="ExternalInput")
    out = nc.dram_tensor("out", [M_LOC, DOUT], mybir.dt.float16,
                         kind="ExternalOutput")

    with tile.TileContext(nc) as tc:
        with tc.tile_pool(name="w8p", bufs=3) as w8p, \
             tc.tile_pool(name="x8p", bufs=1) as x8p, \
             tc.tile_pool(name="outp", bufs=8) as outp, \
             tc.tile_pool(name="cst", bufs=1) as cst, \
             tc.tile_pool(name="psum", bufs=8, space="PSUM") as psump:

            # resident fp8 x: m=0,1 in one combined tile (loaded in 2 big
            # DMAs during the ramp), the rest as per-m tiles
            x01 = x8p.tile([P, WARM_M, K_SUB, P], mybir.dt.float8e5,
                           tag="x01", name="x01")
            x8 = {m: x8p.tile([P, K_SUB, P], mybir.dt.float8e5,
                              tag=f"x8_{m}", name=f"x8_{m}")
                  for m in range(WARM_M, M_TILES)}

            def xap(m, kc):
                if m < WARM_M:
                    return x01[:, m, 2 * kc:2 * kc + 2, :]
                return x8[m][:, 2 * kc:2 * kc + 2, :]

            w8 = {}

            def load_w(j, splits=None, eng=nc.sync):
                w8[j] = w8p.tile([P, K_SUB, N_TILE], mybir.dt.float8e4,
                                 tag="w8", name=f"w8_{j}")
                for a, b in (splits or [(0, K_SUB)]):
                    eng.dma_start(w8[j][:, a:b, :], wd[j, :, a:b, :])

            def load_x(m, eng=nc.scalar):
                eng.dma_start(x8[m][:], xd[m, :, :, :])

            # ---- prologue DMAs (emission order = per-queue FIFO order).
            # HWDGE arbitration favors the sync ring: concurrent scalar
            # transfers get starved to ~40-140GB/s while sync runs at
            # ~220-360GB/s (measured). So ALL ramp-critical data (w0 + the
            # combined x block for m=0..3) rides sync, interleaved in
            # consumption order; scalar carries only slack-tolerant loads.
            # The bias broadcast is a slow replicating DMA (~8-10us of SDMA
            # time); psum depth 8 lets the first eviction wait for it until
            # ~6 steady groups in.
            RAMP = [(0, 2), (2, 4), (4, 8), (8, 16), (16, 24), (24, 32)]
            w8[0] = w8p.tile([P, K_SUB, N_TILE], mybir.dt.float8e4,
                             tag="w8", name="w8_0")
            for a, b in RAMP:
                # x before w: the LDWEIGHTS (stationary = x) precedes each
                # matmul, so x chunks are needed marginally earlier
                nc.sync.dma_start(x01[:, :, a:b, :], xd01[:, :, a:b, :])
                nc.sync.dma_start(w8[0][:, a:b, :], wd[0, :, a:b, :])
            load_x(4)
            load_x(5)
            load_x(6)
            load_x(7)
            bias_rep = cst.tile([P, DOUT], mybir.dt.float16)
            nc.scalar.dma_start(bias_rep[:],
                                bvec.ap()[None, :].to_broadcast((P, DOUT)))
            load_x(8)
            load_x(9)
            load_w(1, splits=[(0, 16), (16, 32)], eng=nc.scalar)
            for m in range(10, M_TILES):
                load_x(m)

            psum = {}

            def mm(j, m, kc):
                nc.tensor.matmul(
                    psum[m][:],
                    xap(m, kc),
                    w8[j][:, 2 * kc:2 * kc + 2, :],
                    start=(kc == 0),
                    stop=(kc == K_CHUNKS - 1),
                    perf_mode=mybir.MatmulPerfMode.DoubleRow,
                )

            def evict(j, m, split=False):
                if not split:
                    ob = outp.tile([P, N_TILE], mybir.dt.float16, tag="ob",
                                   name=f"ob_{j}_{m}")
                    nc.vector.tensor_add(
                        ob[:], psum[m][:],
                        bias_rep[:, j * N_TILE:(j + 1) * N_TILE])
                    nc.sync.dma_start(
                        out[m * P:(m + 1) * P,
                            j * N_TILE:(j + 1) * N_TILE], ob[:])
                    return
                # final group: halve the eviction and store the halves on
                # both HWDGE queues so the closing store latency overlaps
                h = N_TILE // 2
                for c, eng in ((0, nc.scalar), (1, nc.sync)):
                    ob = outp.tile([P, h], mybir.dt.float16, tag="obs",
                                   name=f"ob_{j}_{m}_{c}")
                    nc.vector.tensor_add(
                        ob[:], psum[m][:, c * h:(c + 1) * h],
                        bias_rep[:, j * N_TILE + c * h:
                                 j * N_TILE + (c + 1) * h])
                    eng.dma_start(
                        out[m * P:(m + 1) * P,
                            j * N_TILE + c * h:j * N_TILE + (c + 1) * h],
                        ob[:])

            def do_group(j, m):
                psum[m] = psump.tile([P, N_TILE], mybir.dt.float32, tag="ps",
                                     name=f"ps_{j}_{m}")
                for kc in range(K_CHUNKS):
                    mm(j, m, kc)
                evict(j, m,
                      split=(j == N_TILES - 1 and m == M_TILES - 1))

            # ---- warm-up: column 0, m=0..3 k-chunk-outer so each arriving
            # w0/x chunk unlocks WARM_M matmuls ----
            for m in range(WARM_M):
                psum[m] = psump.tile([P, N_TILE], mybir.dt.float32, tag="ps",
                                     name=f"ps_0_{m}")
            for kc in range(K_CHUNKS):
                for m in range(WARM_M):
                    mm(0, m, kc)
            for m in range(WARM_M):
                evict(0, m)

            # ---- steady state: column-major, group-serial; w tiles
            # prefetched one column ahead as single 2MB DMAs ----
            for m in range(WARM_M, M_TILES):
                if m == 4:
                    load_w(2)
                do_group(0, m)
            for j in range(1, N_TILES):
                for m in range(M_TILES):
                    if m == 0 and j + 2 < N_TILES:
                        load_w(j + 2)
                    do_group(j, m)

    nc.compile()
    _cached_nc = nc
    return nc


def make_in_maps(x, weight, bias):
    x = np.asarray(x)
    weight = np.asarray(weight)
    bias = np.ascontiguousarray(np.asarray(bias))
    assert x.dtype == np.float16 and weight.dtype == np.float16

    # quantize exactly as the reference does (RNE casts)
    x8 = x.astype(ml_dtypes.float8_e5m2)
    w8 = weight.astype(ml_dtypes.float8_e4m3fn)

    # weight [DOUT, DIN] -> [j, ki, ko, n]: wd[j,ki,ko,n] = w8[j*512+n,
    # ko*128+ki] (i.e. weight.T in per-tile K-major blocks)
    wd = np.ascontiguousarray(
        w8.reshape(N_TILES, N_TILE, K_SUB, P).transpose(0, 3, 2, 1))

    xf = x8.reshape(M_TOTAL, DIN)
    in_maps = []
    for c in range(NCORES):
        xc = xf[c * M_LOC:(c + 1) * M_LOC]
        # [M_LOC, DIN] -> [m-tile, ki, ko, m]: xd[t,ki,ko,m] = xc[t*128+m,
        # ko*128+ki]
        xd = np.ascontiguousarray(
            xc.reshape(M_TILES, P, K_SUB, P).transpose(0, 3, 2, 1))
        # first two m-tiles also packed as one [ki, t, ko, m] block so the
        # warm-up working set arrives in two large DMAs
        xd01 = np.ascontiguousarray(xd[:WARM_M].transpose(1, 0, 2, 3))
        in_maps.append({"xd01": xd01, "xd": xd, "wd": wd, "bvec": bias})
    return in_maps


def gather_out(results):
    out = np.concatenate([r["out"] for r in results], axis=0)
    return out.reshape(B, S, DOUT)


def kernel(x, weight, bias):
    from concourse.bass_utils import run_bass_kernel_spmd

    nc = _build()
    in_maps = make_in_maps(x, weight, bias)
    res = run_bass_kernel_spmd(nc, in_maps, core_ids=list(range(NCORES)))
    return gather_out(res.results)


# revision 22
# speedup vs baseline: 1.0056x; 1.0056x over previous
"""FP8 GEMM kernel for Trainium2 (8 NeuronCores, SPMD data-parallel over tokens).

Computes: out = fp16( fp32( e5m2(x) @ e4m3(weight.T) ) + bias )
  x      [4, 4096, 4096] fp16
  weight [4096, 4096]    fp16  (out_features, in_features)
  bias   [4096]          fp16
  out    [4, 4096, 4096] fp16

Sharding: token dim (B*S = 16384) split across 8 cores (2048 rows each);
weight + bias replicated. No collectives; host concatenates the outputs.

The host quantizes both operands to fp8 (ml_dtypes RNE — bit-identical to
the reference's own jnp casts) and pre-packs them into per-tile K-major
blocks (`[tile][ki=128][ko=32][free]`), so every device load is a plain
contiguous fp8 HWDGE DMA — no in-flight cast, half the bytes of an fp16
stream.

Per-core kernel (~465us against the 442us fp8 DoubleRow streaming floor):
 - DoubleRow fp8 matmuls (K=256/instr, moving free dim 2x512 at the
   ~216ns/MM streaming floor) accumulate fp32 into PSUM; x (8MB) stays
   resident, w n-tiles stream through a 3-deep pool.
 - Ramp: the chip starts in a low-power state (~half PE clock, degraded
   DMA) for its first ~15us and HWDGE arbitration starves the scalar ring
   while sync is busy, so ALL ramp-critical data (w0 + a combined x block
   for m=0..3) rides the sync queue in consumption order, and the first 4
   m-groups run k-chunk-outer interleaved across 4 PSUM banks so each
   arriving chunk unlocks 4 matmuls and the PE never outruns the chain.
 - Slack-tolerant loads (x4.., w1, bias broadcast) ride scalar; the bias
   broadcast is a slow replicating DMA (~8-10us of SDMA time), placed so
   psum depth 8 absorbs its latency. Output stores + later w tiles ride
   sync.
 - Bias add fused into the PSUM eviction on DVE (its only job). The final
   group's eviction/store is split across both queues to overlap the
   closing HBM-write receipt.
"""

import sys

if "/opt/trn_rl_repo" not in sys.path:
    sys.path.insert(0, "/opt/trn_rl_repo")

import ml_dtypes
import numpy as np

B, S, DIN, DOUT = 4, 4096, 4096, 4096
NCORES = 8
M_TOTAL = B * S              # 16384
M_LOC = M_TOTAL // NCORES    # 2048
P = 128
M_TILES = M_LOC // P         # 16 m-tiles of 128 rows
N_TILE = 512
N_TILES = DOUT // N_TILE     # 8
K_SUB = DIN // P             # 32 k-subtiles of 128
K_CHUNKS = K_SUB // 2        # 16 DoubleRow chunks of 256
WARM_M = 4                   # m-groups interleaved during the w0 ramp

_cached_nc = None


def _build():
    global _cached_nc
    if _cached_nc is not None:
        return _cached_nc

    import concourse.mybir as mybir
    import concourse.tile as tile
    from concourse import bacc

    nc = bacc.Bacc("TRN2", target_bir_lowering=False, debug=False,
                   num_devices=NCORES)

    # host-packed fp8 K-major tile blocks (see make_in_maps)
    xd01 = nc.dram_tensor("xd01", [P, WARM_M, K_SUB, P], mybir.dt.float8e5,
                          kind="ExternalInput")
    xd = nc.dram_tensor("xd", [M_TILES, P, K_SUB, P], mybir.dt.float8e5,
                        kind="ExternalInput")
    wd = nc.dram_tensor("wd", [N_TILES, P, K_SUB, N_TILE], mybir.dt.float8e4,
                        kind="ExternalInput")
    bvec = nc.dram_tensor("bvec", [DOUT], mybir.dt.float16,
                          kind="ExternalInput")
    out = nc.dram_tensor("out", [M_LOC, DOUT], mybir.dt.float16,
                         kind="ExternalOutput")

    with tile.TileContext(nc) as tc:
        with tc.tile_pool(name="w8p", bufs=3) as w8p, \
             tc.tile_pool(name="x8p", bufs=1) as x8p, \
             tc.tile_pool(name="outp", bufs=8) as outp, \
             tc.tile_pool(name="cst", bufs=1) as cst, \
             tc.tile_pool(name="psum", bufs=8, space="PSUM") as psump:

            # resident fp8 x: m=0..3 in one combined ko-major tile (so one
            # ramp DMA delivers a k-slice for all warm groups at once), the
            # rest as per-m tiles
            x01 = x8p.tile([P, WARM_M, K_SUB, P], mybir.dt.float8e5,
                           tag="x01", name="x01")
            x8 = {m: x8p.tile([P, K_SUB, P], mybir.dt.float8e5,
                              tag=f"x8_{m}", name=f"x8_{m}")
                  for m in range(WARM_M, M_TILES)}

            def xap(m, kc):
                if m < WARM_M:
                    return x01[:, m, 2 * kc:2 * kc + 2, :]
                return x8[m][:, 2 * kc:2 * kc + 2, :]

            w8 = {}

            def load_w(j, splits=None, eng=nc.sync):
                w8[j] = w8p.tile([P, K_SUB, N_TILE], mybir.dt.float8e4,
                                 tag="w8", name=f"w8_{j}")
                for a, b in (splits or [(0, K_SUB)]):
                    eng.dma_start(w8[j][:, a:b, :], wd[j, :, a:b, :])

            def load_x(m, eng=nc.scalar):
                eng.dma_start(x8[m][:], xd[m, :, :, :])

            # ---- prologue DMAs (emission order = per-queue FIFO order).
            # HWDGE arbitration favors the sync ring: concurrent scalar
            # transfers get starved to ~40-140GB/s while sync runs at
            # ~220-360GB/s (measured). So ALL ramp-critical data (w0 + the
            # combined x block for m=0..3) rides sync, interleaved in
            # consumption order; scalar carries only slack-tolerant loads.
            # The bias broadcast is a slow replicating DMA (~8-10us of SDMA
            # time); psum depth 8 lets the first eviction wait for it until
            # ~6 steady groups in.
            RAMP = [(0, 2), (2, 4), (4, 8), (8, 16), (16, 24), (24, 32)]
            w8[0] = w8p.tile([P, K_SUB, N_TILE], mybir.dt.float8e4,
                             tag="w8", name="w8_0")
            for a, b in RAMP:
                # smallest transfer first: the very first DMA pays an extra
                # cold-start latency that scales with its size
                nc.sync.dma_start(w8[0][:, a:b, :], wd[0, :, a:b, :])
                nc.sync.dma_start(x01[:, :, a:b, :], xd01[:, :, a:b, :])
            load_x(4)
            load_x(5)
            load_x(6)
            load_x(7)
            bias_rep = cst.tile([P, DOUT], mybir.dt.float16)
            nc.scalar.dma_start(bias_rep[:],
                                bvec.ap()[None, :].to_broadcast((P, DOUT)))
            load_x(8)
            load_x(9)
            load_w(1, splits=[(0, 16), (16, 32)], eng=nc.scalar)
            for m in range(10, M_TILES):
                load_x(m)

            psum = {}

            def mm(j, m, kc):
                nc.tensor.matmul(
                    psum[m][:],
                    xap(m, kc),
                    w8[j][:, 2 * kc:2 * kc + 2, :],
                    start=(kc == 0),
                    stop=(kc == K_CHUNKS - 1),
                    perf_mode=mybir.MatmulPerfMode.DoubleRow,
                )

            def evict(j, m, split=False):
                if not split:
                    ob = outp.tile([P, N_TILE], mybir.dt.float16, tag="ob",
                                   name=f"ob_{j}_{m}")
                    nc.vector.tensor_add(
                        ob[:], psum[m][:],
                        bias_rep[:, j * N_TILE:(j + 1) * N_TILE])
                    nc.sync.dma_start(
                        out[m * P:(m + 1) * P,
                            j * N_TILE:(j + 1) * N_TILE], ob[:])
                    return
                # final group: halve the eviction and store the halves on
                # both HWDGE queues so the closing store latency overlaps
                h = N_TILE // 2
                for c, eng in ((0, nc.scalar), (1, nc.sync)):
                    ob = outp.tile([P, h], mybir.dt.float16, tag="obs",
                                   name=f"ob_{j}_{m}_{c}")
                    nc.vector.tensor_add(
                        ob[:], psum[m][:, c * h:(c + 1) * h],
                        bias_rep[:, j * N_TILE + c * h:
                                 j * N_TILE + (c + 1) * h])
                    eng.dma_start(
                        out[m * P:(m + 1) * P,
                            j * N_TILE + c * h:j * N_TILE + (c + 1) * h],
                        ob[:])

            def do_group(j, m):
                psum[m] = psump.tile([P, N_TILE], mybir.dt.float32, tag="ps",
                                     name=f"ps_{j}_{m}")
                for kc in range(K_CHUNKS):
                    mm(j, m, kc)
                evict(j, m,
                      split=(j == N_TILES - 1 and m == M_TILES - 1))

            # ---- warm-up: column 0, m=0..3 k-chunk-outer so each arriving
            # w0/x chunk unlocks WARM_M matmuls ----
            for m in range(WARM_M):
                psum[m] = psump.tile([P, N_TILE], mybir.dt.float32, tag="ps",
                                     name=f"ps_0_{m}")
            for kc in range(K_CHUNKS):
                for m in range(WARM_M):
                    mm(0, m, kc)
            for m in range(WARM_M):
                evict(0, m)

            # ---- steady state: column-major, group-serial; w tiles
            # prefetched one column ahead as single 2MB DMAs ----
            for m in range(WARM_M, M_TILES):
                if m == 4:
                    load_w(2)
                do_group(0, m)
            for j in range(1, N_TILES):
                for m in range(M_TILES):
                    if m == 0 and j + 2 < N_TILES:
                        load_w(j + 2)
                    do_group(j, m)

    nc.compile()
    _cached_nc = nc
    return nc


def make_in_maps(x, weight, bias):
    x = np.asarray(x)
    weight = np.asarray(weight)
    bias = np.ascontiguousarray(np.asarray(bias))
    assert x.dtype == np.float16 and weight.dtype == np.float16

    # quantize exactly as the reference does (RNE casts)
    x8 = x.astype(ml_dtypes.float8_e5m2)
    w8 = weight.astype(ml_dtypes.float8_e4m3fn)

    # weight [DOUT, DIN] -> [j, ki, ko, n]: wd[j,ki,ko,n] = w8[j*512+n,
    # ko*128+ki] (i.e. weight.T in per-tile K-major blocks)
    wd = np.ascontiguousarray(
        w8.reshape(N_TILES, N_TILE, K_SUB, P).transpose(0, 3, 2, 1))

    xf = x8.reshape(M_TOTAL, DIN)
    in_maps = []
    for c in range(NCORES):
        xc = xf[c * M_LOC:(c + 1) * M_LOC]
        # [M_LOC, DIN] -> [m-tile, ki, ko, m]: xd[t,ki,ko,m] = xc[t*128+m,
        # ko*128+ki]
        xd = np.ascontiguousarray(
            xc.reshape(M_TILES, P, K_SUB, P).transpose(0, 3, 2, 1))
        # first WARM_M m-tiles also packed as one [ki, t, ko, m] block so
        # each ramp DMA delivers a k-slice for all warm groups at once
        xd01 = np.ascontiguousarray(xd[:WARM_M].transpose(1, 0, 2, 3))
        in_maps.append({"xd01": xd01, "xd": xd, "wd": wd, "bvec": bias})
    return in_maps


def gather_out(results):
    out = np.concatenate([r["out"] for r in results], axis=0)
    return out.reshape(B, S, DOUT)


def kernel(x, weight, bias):
    from concourse.bass_utils import run_bass_kernel_spmd

    nc = _build()
    in_maps = make_in_maps(x, weight, bias)
    res = run_bass_kernel_spmd(nc, in_maps, core_ids=list(range(NCORES)))
    return gather_out(res.results)
